# revision 1
# baseline (speedup 1.0000x reference)
"""Causal self-attention on 8 Trainium2 NeuronCores.

Sharding (data + head parallel): core c handles batch b = c // 4 and the
4 heads [4g, 4g+4) where g = c % 4.  Each core projects q/k/v for its
heads (weights pre-sliced + pre-transposed on host), runs causal
attention, then the 4 cores of each batch AllGather the per-head
attention outputs (hd-major fp16) and each computes a disjoint
256-channel column slice of the output projection.

Pipeline notes:
- fp16 data path, fp32 PSUM accumulation.
- Attention runs in 512-column q-chunks; both heads of a pair share one
  score tile (bank-aligned halves) so each j-step needs a single exp.
- PSUM budget (8 banks): score tile 2 banks x 2 bufs, two 1-bank
  attention accumulators, one 2-bank projection accumulator.  The spare
  projection accumulator lets q/k projection for pair 1 and the output
  projection interleave with the ACT-bound attention loop, keeping the
  tensor engine dense (HAM stays un-throttled).
- AllGathers go per (pair, 1024 columns): 4 small collectives that
  overlap attention; gathered rows are prefetched to SBUF immediately.

Layouts per core:
  xT    (1024, 2048)  x[b].T                       (d on partitions)
  wqkT  (1024, 512)   [ (Wq[rows]/8).T | Wk[rows].T ]
  wvT   (1024, 256)   Wv[rows].T
  woT   (1024, 256)   Wo[rows].T with rows permuted to the AllGather
                      order: [pair p=0: rank r: heads 4r,4r+1] then
                      [pair p=1: rank r: heads 4r+2,4r+3]
  mask  (128, 128)    upper-triangular ones (k <= q)
  outT  (256, 2048)   out[b][:, cols].T
"""

import numpy as np

B, S, D, H = 2, 2048, 1024, 16
HD = D // H              # 64
NCORES = 8
GROUP = 4                # cores per batch
LHEADS = 4               # heads per core
LCH = LHEADS * HD        # 256 local channels
KT = D // 128            # 8 contraction tiles
ST = S // 128            # 16 sequence tiles
PAIRS = 2                # head pairs per core
CHUNK = 512              # q columns per attention pass
NCH = S // CHUNK         # 4
GATH = 1024              # columns per collective
NHB = S // GATH          # 2 gather blocks

_CACHE = {}


def _f16(a):
    return np.ascontiguousarray(a, dtype=np.float16)


def _build():
    import concourse.bacc as bacc
    import concourse.mybir as mybir
    import concourse.tile as tile

    f32 = mybir.dt.float32
    f16 = mybir.dt.float16
    Exp = mybir.ActivationFunctionType.Exp

    nc = bacc.Bacc(num_devices=NCORES)
    xT = nc.dram_tensor("xT", [D, S], f16, kind="ExternalInput")
    wqkT = nc.dram_tensor("wqkT", [D, 2 * LCH], f16, kind="ExternalInput")
    wvT = nc.dram_tensor("wvT", [D, LCH], f16, kind="ExternalInput")
    woT = nc.dram_tensor("woT", [D, LCH], f16, kind="ExternalInput")
    mask = nc.dram_tensor("mask", [128, 128], f16, kind="ExternalInput")
    outT = nc.dram_tensor("outT", [LCH, S], f32, kind="ExternalOutput")

    RG = [[0, 1, 2, 3], [4, 5, 6, 7]]

    with tile.TileContext(nc, num_cores=NCORES) as tc:
        with (
            tc.tile_pool(name="const", bufs=1) as const,
            tc.tile_pool(name="qkv", bufs=1) as qkv,
            tc.tile_pool(name="psum", bufs=1, space="PSUM") as psum,
            tc.tile_pool(name="dram", bufs=1, space="DRAM") as dram,
            tc.tile_pool(name="work", bufs=1) as work,
            tc.tile_pool(name="proj", bufs=1) as projp,
            tc.tile_pool(name="agp", bufs=1) as agp,
        ):
            mask_sb = const.tile([128, 128], f16)
            nc.sync.dma_start(mask_sb[:], mask[:])
            ones4 = const.tile([128, LHEADS], f32)
            nc.vector.memset(ones4[:], 1.0)

            cc_in = [[dram.tile([128, GATH], f16, name=f"ccin{p}{hb}")
                      for hb in range(NHB)] for p in range(PAIRS)]
            cc_out = [[dram.tile([GROUP * 128, GATH], f16, name=f"ccout{p}{hb}")
                       for hb in range(NHB)] for p in range(PAIRS)]

            qt = qkv.tile([128, PAIRS, S], f16)
            kt = qkv.tile([128, PAIRS, S], f16)
            v = qkv.tile([128, ST, LHEADS, 65], f16)

            # ---------------- input loads ----------------
            xt, wqk, wv = [], [], []
            for k in range(KT):
                tx = projp.tile([128, S], f16, name=f"xt{k}")
                nc.sync.dma_start(tx[:], xT[128 * k:128 * k + 128, :])
                xt.append(tx)
                tw = projp.tile([128, 2 * LCH], f16, name=f"wqk{k}")
                nc.sync.dma_start(tw[:], wqkT[128 * k:128 * k + 128, :])
                wqk.append(tw)
            for k in range(KT):
                tv = projp.tile([128, LCH], f16, name=f"wv{k}")
                nc.sync.dma_start(tv[:], wvT[128 * k:128 * k + 128, :])
                wv.append(tv)
            wo = projp.tile([128, KT, LCH], f16)
            nc.sync.dma_start(wo[:], woT[:].rearrange("(k p) n -> p k n", p=128))

            def qk_proj(m):
                # m: 0,1 = q pair 0/1; 2,3 = k pair 0/1
                dst = qt if m < 2 else kt
                for half in range(2):
                    pp = psum.tile([128, GATH], f32, tag="pp", name=f"qk{m}{half}")
                    for k in range(KT):
                        for c2 in range(2):
                            o = GATH * half + 512 * c2
                            nc.tensor.matmul(
                                pp[:, 512 * c2:512 * c2 + 512],
                                wqk[k][:, 128 * m:128 * m + 128],
                                xt[k][:, o:o + 512],
                                start=(k == 0), stop=(k == KT - 1))
                    nc.vector.tensor_copy(
                        dst[:, m % 2, GATH * half:GATH * half + GATH], pp[:])

            def v_proj(j):
                vps = psum.tile([128, LCH], f32, tag=("at" if j % 2 == 0 else "pp"),
                                name=f"v{j}")
                for k in range(KT):
                    nc.tensor.matmul(
                        vps[:], xt[k][:, 128 * j:128 * j + 128], wv[k][:],
                        start=(k == 0), stop=(k == KT - 1))
                nc.vector.tensor_copy(
                    v[:, j, :, 64:65], ones4[:].rearrange("p (h o) -> p h o", o=1))
                nc.vector.tensor_copy(
                    v[:, j, :, 0:64], vps[:].rearrange("p (h e) -> p h e", h=LHEADS))

            ag = {}
            att_last = [None]
            ccin_last = [None]

            def stage_chunk(p, c, attps):
                """Normalize chunk c's accumulators and ship to the collective
                buffer; AllGather once a 1024-col block is done."""
                hb, sub = c // 2, c % 2
                for h in range(2):
                    asb = work.tile([65, CHUNK], f32, tag=f"asb{h}", bufs=2,
                                    name=f"asb{p}{c}{h}")
                    nc.vector.tensor_copy(asb[:], attps[:, 512 * h:512 * h + CHUNK])
                    rc = work.tile([65, CHUNK], f32, tag="rc", bufs=2,
                                   name=f"rc{p}{c}{h}")
                    nc.vector.reciprocal(rc[64:65, :], asb[64:65, :])
                    rc0 = work.tile([1, CHUNK], f32, tag="rc0", bufs=2,
                                    name=f"rc0{p}{c}{h}")
                    nc.sync.dma_start(rc0[0:1, :], rc[64:65, :])
                    bc = work.tile([64, CHUNK], f32, tag="bc", bufs=2,
                                   name=f"bc{p}{c}{h}")
                    nc.gpsimd.partition_broadcast(bc[:], rc0[0:1, :])
                    ao = work.tile([64, CHUNK], f16, tag="ao", bufs=2,
                                   name=f"ao{p}{c}{h}")
                    nc.vector.tensor_mul(ao[:, :], asb[0:64, :], bc[:, :])
                    ccin_last[0] = nc.sync.dma_start(
                        cc_in[p][hb][64 * h:64 * h + 64,
                                     CHUNK * sub:CHUNK * sub + CHUNK], ao[:, :])
                if sub == 1:
                    nc.gpsimd.collective_compute(
                        "AllGather", mybir.AluOpType.bypass, replica_groups=RG,
                        ins=[cc_in[p][hb][:]], outs=[cc_out[p][hb][:]])

            def attn_chunk(p, c):
                q0 = CHUNK * c
                nj = 4 * c + 4
                attps = psum.tile([65, 2 * CHUNK], f32,
                                  tag=("at" if c % 2 == 0 else "pp"),
                                  name=f"att{p}{c}")
                for j in range(nj):
                    qs = max(q0, 128 * j)
                    n = q0 + CHUNK - qs
                    off = qs - q0
                    sc = psum.tile([128, 1024], f32, tag="sc", bufs=2,
                                   name=f"sc{p}{c}{j}")
                    for h in range(2):
                        pb = 64 * h
                        nc.tensor.matmul(
                            sc[:, 512 * h:512 * h + n],
                            kt[pb:pb + 64, p, 128 * j:128 * j + 128],
                            qt[pb:pb + 64, p, qs:qs + n],
                            start=True, stop=True)
                    ex = work.tile([128, 1024], f16, tag="ex", bufs=3,
                                   name=f"ex{p}{c}{j}")
                    nc.scalar.activation(
                        ex[:].rearrange("q (t x) -> q t x", t=2)[:, :, 0:n],
                        sc[:].rearrange("q (t x) -> q t x", t=2)[:, :, 0:n],
                        Exp)
                    if qs == 128 * j:  # diagonal tile: causal mask
                        for h in range(2):
                            nc.vector.tensor_mul(
                                ex[:, 512 * h:512 * h + 128],
                                ex[:, 512 * h:512 * h + 128], mask_sb[:])
                    for h in range(2):
                        att_last[0] = nc.tensor.matmul(
                            attps[:, 512 * h + off:512 * h + CHUNK],
                            v[:, j, 2 * p + h, :],
                            ex[:, 512 * h:512 * h + n],
                            start=(j == 0), stop=(j == nj - 1))
                stage_chunk(p, c, attps)

            def out_proj(hb):
                # The scheduler's cost model doesn't know collective latency;
                # pin the gather prefetch (and hence the matmuls) after the
                # last attention instruction so a long AllGather wait can't
                # stall the in-order engine streams mid-attention.
                for p in range(PAIRS):
                    for r in range(GROUP):
                        t = agp.tile([128, GATH], f16, name=f"ag{p}{hb}{r}")
                        dma = nc.sync.dma_start(
                            t[:], cc_out[p][hb][128 * r:128 * r + 128, :])
                        if ccin_last[0] is not None:
                            tile.add_dep_helper(
                                dma.ins, ccin_last[0].ins, sync=True,
                                reason="gather prefetch after all staging")
                        ag[(p, hb, r)] = t
                for ct in range(2):
                    pp = psum.tile([128, GATH], f32, tag="pp", name=f"op{hb}{ct}")
                    for k in range(KT):
                        for c2 in range(2):
                            nc.tensor.matmul(
                                pp[:, 512 * c2:512 * c2 + 512],
                                wo[:, k, 128 * ct:128 * ct + 128],
                                ag[(k // 4, hb, k % 4)][:, 512 * c2:512 * c2 + 512],
                                start=(k == 0), stop=(k == KT - 1))
                    ot = agp.tile([128, GATH], f32, tag=f"ot{ct}", bufs=2,
                                  name=f"ot{hb}{ct}")
                    nc.scalar.copy(ot[:], pp[:])
                    nc.sync.dma_start(
                        outT[128 * ct:128 * ct + 128,
                             GATH * hb:GATH * hb + GATH], ot[:])

            # ---------------- schedule ----------------
            qk_proj(0)            # pair-0 q
            qk_proj(2)            # pair-0 k
            for j in range(8):
                v_proj(j)
            attn_chunk(0, 0)
            attn_chunk(0, 1)
            for j in range(8, ST):
                v_proj(j)
            qk_proj(1)            # pair-1 q
            qk_proj(3)            # pair-1 k
            attn_chunk(0, 2)
            attn_chunk(0, 3)
            for c in range(NCH):
                attn_chunk(1, c)
            out_proj(0)
            out_proj(1)

    nc.compile()
    return nc


def _gather_perm():
    """d-channel permutation matching the AllGather layout."""
    perm = []
    for p in range(PAIRS):
        for r in range(GROUP):
            for h in range(2):
                head = 4 * r + 2 * p + h
                perm.extend(range(HD * head, HD * head + HD))
    return np.array(perm)


def _shard_inputs(x, Wq, Wk, Wv, Wo):
    x = np.asarray(x, dtype=np.float32)
    Wq = np.asarray(Wq, dtype=np.float32)
    Wk = np.asarray(Wk, dtype=np.float32)
    Wv = np.asarray(Wv, dtype=np.float32)
    Wo = np.asarray(Wo, dtype=np.float32)
    mask = np.triu(np.ones((128, 128), dtype=np.float16))
    perm = _gather_perm()
    in_maps = []
    for c in range(NCORES):
        b, g = c // GROUP, c % GROUP
        rows = slice(LCH * g, LCH * g + LCH)
        in_maps.append({
            "xT": _f16(x[b].T),
            "wqkT": _f16(np.concatenate([Wq[rows] / 8.0, Wk[rows]], axis=0).T),
            "wvT": _f16(Wv[rows].T),
            "woT": _f16(Wo[rows].T[perm, :]),
            "mask": mask,
        })
    return in_maps


def kernel(x, Wq, Wk, Wv, Wo):
    from concourse.bass_utils import run_bass_kernel_spmd

    if "nc" not in _CACHE:
        _CACHE["nc"] = _build()
    nc = _CACHE["nc"]
    in_maps = _shard_inputs(x, Wq, Wk, Wv, Wo)
    res = run_bass_kernel_spmd(nc, in_maps, core_ids=list(range(NCORES)))
    _CACHE["last_results"] = res
    out = np.empty((B, S, D), dtype=np.float32)
    for c in range(NCORES):
        b, g = c // GROUP, c % GROUP
        out[b][:, LCH * g:LCH * g + LCH] = res.results[c]["outT"].T
    return out



# revision 4
# speedup vs baseline: 1.0283x; 1.0283x over previous
"""Causal self-attention on 8 Trainium2 NeuronCores.

Sharding (data + head parallel): core c handles batch b = c // 4 and the
4 heads [4g, 4g+4) where g = c % 4.  Each core projects q/k/v for its
heads (weights pre-sliced + pre-transposed on host), runs causal
attention, then the 4 cores of each batch AllGather the per-head
attention outputs (hd-major fp16) and each computes a disjoint
256-channel column slice of the output projection.

Schedule notes (v2):
- The two head-pairs are interleaved chunk-by-chunk so both pairs'
  normalized outputs for sequence block hb are staged together; one
  AllGather per 1024-column block (covering both pairs) fires at ~50%
  and ~100% of the attention phase, overlapping the collective with
  compute instead of serializing 4 gathers at the end.
- A tiny warmup AllGather at kernel start absorbs the CC-stream
  first-collective ramp cost.
- Softmax normalization: reciprocal_approx_fast (custom DVE op, ~5x
  faster than InstReciprocal) reads the denominator row straight from
  PSUM; the numerator x (1/den) multiply also reads PSUM directly (no
  staging copy).
- fp16 data path, fp32 PSUM accumulation.
- PSUM budget (8 banks): score tile 2 banks x 2 bufs, "at"/"pp"
  accumulators 2 banks each shared by attention chunks (alternating)
  and the q/k/v/output projections.

Layouts per core:
  xT    (1024, 2048)  x[b].T                       (d on partitions)
  wqkT  (1024, 512)   [ (Wq[rows]/8).T | Wk[rows].T ]
  wvT   (1024, 256)   Wv[rows].T
  woT   (1024, 256)   Wo[rows].T with rows permuted to the AllGather
                      order: [pair p=0: rank r: heads 4r,4r+1] then
                      [pair p=1: rank r: heads 4r+2,4r+3]
  mask  (128, 128)    upper-triangular ones (k <= q)
  outT  (256, 2048)   out[b][:, cols].T
"""

import numpy as np

B, S, D, H = 2, 2048, 1024, 16
HD = D // H              # 64
NCORES = 8
GROUP = 4                # cores per batch
LHEADS = 4               # heads per core
LCH = LHEADS * HD        # 256 local channels
KT = D // 128            # 8 contraction tiles
ST = S // 128            # 16 sequence tiles
PAIRS = 2                # head pairs per core
CHUNK = 512              # q columns per attention pass
NCH = S // CHUNK         # 4
GATH = 1024              # columns per collective
NHB = S // GATH          # 2 gather blocks

_CACHE = {}


def _f16(a):
    return np.ascontiguousarray(a, dtype=np.float16)


def _build():
    import concourse.bacc as bacc
    import concourse.mybir as mybir
    import concourse.tile as tile

    f32 = mybir.dt.float32
    f16 = mybir.dt.float16
    Exp = mybir.ActivationFunctionType.Exp

    nc = bacc.Bacc(num_devices=NCORES)
    xT = nc.dram_tensor("xT", [D, S], f16, kind="ExternalInput")
    wqkT = nc.dram_tensor("wqkT", [D, 2 * LCH], f16, kind="ExternalInput")
    wvT = nc.dram_tensor("wvT", [D, LCH], f16, kind="ExternalInput")
    woT = nc.dram_tensor("woT", [D, LCH], f16, kind="ExternalInput")
    mask = nc.dram_tensor("mask", [128, 128], f16, kind="ExternalInput")
    outT = nc.dram_tensor("outT", [LCH, S], f32, kind="ExternalOutput")

    RG = [[0, 1, 2, 3], [4, 5, 6, 7]]

    with tile.TileContext(nc, num_cores=NCORES) as tc:
        with (
            tc.tile_pool(name="const", bufs=1) as const,
            tc.tile_pool(name="qkv", bufs=1) as qkv,
            tc.tile_pool(name="psum", bufs=1, space="PSUM") as psum,
            tc.tile_pool(name="dram", bufs=1, space="DRAM") as dram,
            tc.tile_pool(name="work", bufs=1) as work,
            tc.tile_pool(name="proj", bufs=1) as projp,
            tc.tile_pool(name="agp", bufs=1) as agp,
        ):
            mask_sb = const.tile([128, 128], f16)
            nc.sync.dma_start(mask_sb[:], mask[:])
            ones4 = const.tile([128, LHEADS], f32)
            nc.vector.memset(ones4[:], 1.0)

            # per gather-block collective buffers: both pairs side by side
            # (cols p*GATH + sub*CHUNK + [0,512) for pair p, chunk-sub sub)
            cc_in = [dram.tile([128, 2 * GATH], f16, name=f"ccin{hb}")
                     for hb in range(NHB)]
            cc_out = [dram.tile([GROUP * 128, 2 * GATH], f16, name=f"ccout{hb}")
                      for hb in range(NHB)]

            # warmup collective: absorbs CC-stream first-collective ramp
            warm_in = dram.tile([128, 8], f16, name="warm_in")
            warm_out = dram.tile([GROUP * 128, 8], f16, name="warm_out")
            nc.sync.dma_start(warm_in[:], mask[:, 0:8])
            nc.gpsimd.collective_compute(
                "AllGather", mybir.AluOpType.bypass, replica_groups=RG,
                ins=[warm_in[:]], outs=[warm_out[:]])

            qt = qkv.tile([128, PAIRS, S], f16)
            kt = qkv.tile([128, PAIRS, S], f16)
            v = qkv.tile([128, ST, LHEADS, 65], f16)

            # ---------------- input loads (one DMA per tensor) ----------------
            xt = projp.tile([128, KT, S], f16)
            nc.sync.dma_start(xt[:], xT[:].rearrange("(k p) s -> p k s", p=128))
            wqk = projp.tile([128, KT, 2 * LCH], f16)
            nc.sync.dma_start(wqk[:], wqkT[:].rearrange("(k p) n -> p k n", p=128))
            wv = projp.tile([128, KT, LCH], f16)
            nc.sync.dma_start(wv[:], wvT[:].rearrange("(k p) n -> p k n", p=128))
            wo = projp.tile([128, KT, LCH], f16)
            nc.sync.dma_start(wo[:], woT[:].rearrange("(k p) n -> p k n", p=128))

            def qk_proj(m):
                # m: 0,1 = q pair 0/1; 2,3 = k pair 0/1
                dst = qt if m < 2 else kt
                for half in range(2):
                    pp = psum.tile([128, GATH], f32,
                                   tag=("pp" if half == 0 else "at"),
                                   name=f"qk{m}{half}")
                    for k in range(KT):
                        for c2 in range(2):
                            o = GATH * half + 512 * c2
                            nc.tensor.matmul(
                                pp[:, 512 * c2:512 * c2 + 512],
                                wqk[:, k, 128 * m:128 * m + 128],
                                xt[:, k, o:o + 512],
                                start=(k == 0), stop=(k == KT - 1))
                    nc.vector.tensor_copy(
                        dst[:, m % 2, GATH * half:GATH * half + GATH], pp[:])

            def v_proj(j):
                vps = psum.tile([128, LCH], f32, tag=("at" if j % 2 == 0 else "pp"),
                                name=f"v{j}")
                for k in range(KT):
                    nc.tensor.matmul(
                        vps[:], xt[:, k, 128 * j:128 * j + 128], wv[:, k, :],
                        start=(k == 0), stop=(k == KT - 1))
                nc.vector.tensor_copy(
                    v[:, j, :, 64:65], ones4[:].rearrange("p (h o) -> p h o", o=1))
                nc.vector.tensor_copy(
                    v[:, j, :, 0:64], vps[:].rearrange("p (h e) -> p h e", h=LHEADS))

            ag = {}
            att_last = [None]
            ccin_last = [None]

            def stage_chunk(p, c, attps):
                """Normalize chunk c's accumulators straight out of PSUM and
                ship to the collective buffer."""
                hb, sub = c // 2, c % 2
                # reciprocal of the 1024 denominators: move the single PSUM
                # row out to SBUF, reshape across 64 partitions by DMA (DVE
                # reciprocal is ~7.8ns/elem serial per partition), invert,
                # and reshape back to a row for the partition broadcast.
                den = work.tile([65, 2 * CHUNK], f32, tag="den", bufs=2,
                                name=f"den{p}{c}")
                nc.vector.tensor_copy(den[64:65, :], attps[64:65, :])
                rcs = work.tile([64, 16], f32, tag="rcs", bufs=2,
                                name=f"rcs{p}{c}")
                nc.sync.dma_start(rcs[:], den[64:65, :])
                rcr = work.tile([64, 16], f32, tag="rcr", bufs=2,
                                name=f"rcr{p}{c}")
                nc.vector.reciprocal(rcr[:], rcs[:])
                rc0 = work.tile([1, 2 * CHUNK], f32, tag="rc0", bufs=2,
                                name=f"rc0{p}{c}")
                nc.sync.dma_start(rc0[0:1, :], rcr[:])
                for h in range(2):
                    bc = work.tile([64, CHUNK], f32, tag=f"bc{h}", bufs=2,
                                   name=f"bc{p}{c}{h}")
                    nc.gpsimd.partition_broadcast(
                        bc[:], rc0[0:1, CHUNK * h:CHUNK * h + CHUNK])
                    ao = work.tile([64, CHUNK], f16, tag=f"ao{h}", bufs=2,
                                   name=f"ao{p}{c}{h}")
                    nc.vector.tensor_mul(
                        ao[:, :], attps[0:64, CHUNK * h:CHUNK * h + CHUNK],
                        bc[:, :])
                    o = GATH * p + CHUNK * sub
                    ccin_last[0] = nc.sync.dma_start(
                        cc_in[hb][64 * h:64 * h + 64, o:o + CHUNK], ao[:, :])

            def gather(hb):
                nc.gpsimd.collective_compute(
                    "AllGather", mybir.AluOpType.bypass, replica_groups=RG,
                    ins=[cc_in[hb][:]], outs=[cc_out[hb][:]])

            def attn_chunk(p, c, gc):
                q0 = CHUNK * c
                nj = 4 * c + 4
                attps = psum.tile([65, 2 * CHUNK], f32,
                                  tag=("at" if gc % 2 == 0 else "pp"),
                                  name=f"att{p}{c}")
                for j in range(nj):
                    qs = max(q0, 128 * j)
                    n = q0 + CHUNK - qs
                    off = qs - q0
                    sc = psum.tile([128, 1024], f32, tag="sc", bufs=2,
                                   name=f"sc{p}{c}{j}")
                    for h in range(2):
                        pb = 64 * h
                        nc.tensor.matmul(
                            sc[:, 512 * h:512 * h + n],
                            kt[pb:pb + 64, p, 128 * j:128 * j + 128],
                            qt[pb:pb + 64, p, qs:qs + n],
                            start=True, stop=True)
                    ex = work.tile([128, 1024], f16, tag="ex", bufs=3,
                                   name=f"ex{p}{c}{j}")
                    if n == CHUNK:
                        nc.scalar.activation(ex[:, :], sc[:, :], Exp)
                    else:
                        nc.scalar.activation(
                            ex[:].rearrange("q (t x) -> q t x", t=2)[:, :, 0:n],
                            sc[:].rearrange("q (t x) -> q t x", t=2)[:, :, 0:n],
                            Exp)
                    if qs == 128 * j:  # diagonal tile: causal mask
                        for h in range(2):
                            nc.vector.tensor_mul(
                                ex[:, 512 * h:512 * h + 128],
                                ex[:, 512 * h:512 * h + 128], mask_sb[:])
                    for h in range(2):
                        att_last[0] = nc.tensor.matmul(
                            attps[:, 512 * h + off:512 * h + CHUNK],
                            v[:, j, 2 * p + h, :],
                            ex[:, 512 * h:512 * h + n],
                            start=(j == 0), stop=(j == nj - 1))
                stage_chunk(p, c, attps)

            def out_proj(hb):
                # Pin the gather prefetch (and hence the matmuls) after the
                # last attention staging DMA so a long AllGather wait can't
                # stall the in-order engine streams mid-attention.
                for r in range(GROUP):
                    t = agp.tile([128, 2 * GATH], f16, name=f"ag{hb}{r}")
                    dma = nc.sync.dma_start(
                        t[:], cc_out[hb][128 * r:128 * r + 128, :])
                    if ccin_last[0] is not None:
                        tile.add_dep_helper(
                            dma.ins, ccin_last[0].ins, sync=True,
                            reason="gather prefetch after all staging")
                    ag[(hb, r)] = t
                for ct in range(2):
                    pp = psum.tile([128, GATH], f32,
                                   tag=("pp" if ct == 0 else "at"),
                                   name=f"op{hb}{ct}")
                    for k in range(KT):
                        for c2 in range(2):
                            nc.tensor.matmul(
                                pp[:, 512 * c2:512 * c2 + 512],
                                wo[:, k, 128 * ct:128 * ct + 128],
                                ag[(hb, k % 4)][:, GATH * (k // 4) + 512 * c2:
                                                GATH * (k // 4) + 512 * c2 + 512],
                                start=(k == 0), stop=(k == KT - 1))
                    ot = agp.tile([128, GATH], f32, tag=f"ot{ct}", bufs=2,
                                  name=f"ot{hb}{ct}")
                    nc.scalar.copy(ot[:], pp[:])
                    nc.sync.dma_start(
                        outT[128 * ct:128 * ct + 128,
                             GATH * hb:GATH * hb + GATH], ot[:])

            # ---------------- schedule ----------------
            qk_proj(0)            # pair-0 q
            qk_proj(2)            # pair-0 k
            for j in range(4):
                v_proj(j)
            qk_proj(1)            # pair-1 q
            qk_proj(3)            # pair-1 k
            attn_chunk(0, 0, 0)
            attn_chunk(1, 0, 1)
            for j in range(4, 8):
                v_proj(j)
            attn_chunk(0, 1, 2)
            attn_chunk(1, 1, 3)
            gather(0)
            for j in range(8, 12):
                v_proj(j)
            attn_chunk(0, 2, 4)
            attn_chunk(1, 2, 5)
            for j in range(12, ST):
                v_proj(j)
            attn_chunk(0, 3, 6)
            attn_chunk(1, 3, 7)
            gather(1)
            out_proj(0)
            out_proj(1)

    nc.compile()
    return nc


def _gather_perm():
    """d-channel permutation matching the AllGather layout."""
    perm = []
    for p in range(PAIRS):
        for r in range(GROUP):
            for h in range(2):
                head = 4 * r + 2 * p + h
                perm.extend(range(HD * head, HD * head + HD))
    return np.array(perm)


def _shard_inputs(x, Wq, Wk, Wv, Wo):
    x = np.asarray(x, dtype=np.float32)
    Wq = np.asarray(Wq, dtype=np.float32)
    Wk = np.asarray(Wk, dtype=np.float32)
    Wv = np.asarray(Wv, dtype=np.float32)
    Wo = np.asarray(Wo, dtype=np.float32)
    mask = np.triu(np.ones((128, 128), dtype=np.float16))
    perm = _gather_perm()
    in_maps = []
    for c in range(NCORES):
        b, g = c // GROUP, c % GROUP
        rows = slice(LCH * g, LCH * g + LCH)
        in_maps.append({
            "xT": _f16(x[b].T),
            "wqkT": _f16(np.concatenate([Wq[rows] / 8.0, Wk[rows]], axis=0).T),
            "wvT": _f16(Wv[rows].T),
            "woT": _f16(Wo[rows].T[perm, :]),
            "mask": mask,
        })
    return in_maps


def kernel(x, Wq, Wk, Wv, Wo):
    from concourse.bass_utils import run_bass_kernel_spmd

    if "nc" not in _CACHE:
        _CACHE["nc"] = _build()
    nc = _CACHE["nc"]
    in_maps = _shard_inputs(x, Wq, Wk, Wv, Wo)
    res = run_bass_kernel_spmd(nc, in_maps, core_ids=list(range(NCORES)))
    _CACHE["last_results"] = res
    out = np.empty((B, S, D), dtype=np.float32)
    for c in range(NCORES):
        b, g = c // GROUP, c % GROUP
        out[b][:, LCH * g:LCH * g + LCH] = res.results[c]["outT"].T
    return out


# revision 5
# speedup vs baseline: 1.0780x; 1.0483x over previous
"""Causal self-attention on 8 Trainium2 NeuronCores.

Sharding (data + head parallel): core c handles batch b = c // 4 and the
4 heads [4g, 4g+4) where g = c % 4.  Each core projects q/k/v for its
heads (weights pre-sliced + pre-transposed on host), runs causal
attention, then the 4 cores of each batch AllGather the per-head
attention outputs (hd-major fp16) and each computes a disjoint
256-channel column slice of the output projection.

Schedule notes (v3):
- The two head-pairs are interleaved chunk-by-chunk; one AllGather per
  512-column chunk (covering both pairs) fires as soon as that chunk is
  normalized on both pairs: collectives overlap attention, and only the
  last chunk's (256KB-in) gather is exposed in the tail.
- Output projection runs per 512-column group as soon as its gather has
  landed, filling tensor-engine slack in the ACT(exp)-paced attention
  loop; only the last group is in the tail.
- q/k projections are emitted in 1024-column halves interleaved with the
  first chunks so the exp stream starts ~20us into the kernel.
- Softmax normalization: the 1024 denominators of a chunk are copied out
  of PSUM as one row, reshaped across 64 partitions by a small DMA,
  inverted with DVE reciprocal (serial-per-partition: ~16 elements each
  instead of 1024), reshaped back, broadcast, and multiplied into the
  PSUM accumulators directly.
- A tiny warmup AllGather at kernel start absorbs the CC-stream
  first-collective ramp and cross-core launch skew.
- fp16 data path, fp32 PSUM accumulation, fp16 output (absmax ~4).
- PSUM budget (8 banks): score tile 2 banks x 2 bufs; "at"/"pp" spaces
  2 banks each, shared by attention accumulators (alternating),
  projection tiles, and output-projection tiles in emission order.

Layouts per core:
  xT    (1024, 2048)  x[b].T                       (d on partitions)
  wqkT  (1024, 512)   [ (Wq[rows]/8).T | Wk[rows].T ]
  wvT   (1024, 256)   Wv[rows].T
  woT   (1024, 256)   Wo[rows].T with rows permuted to the AllGather
                      order: [pair p=0: rank r: heads 4r,4r+1] then
                      [pair p=1: rank r: heads 4r+2,4r+3]
  mask  (128, 128)    upper-triangular ones (k <= q)
  outT  (256, 2048)   out[b][:, cols].T  (fp16)
"""

import numpy as np

B, S, D, H = 2, 2048, 1024, 16
HD = D // H              # 64
NCORES = 8
GROUP = 4                # cores per batch
LHEADS = 4               # heads per core
LCH = LHEADS * HD        # 256 local channels
KT = D // 128            # 8 contraction tiles
ST = S // 128            # 16 sequence tiles
PAIRS = 2                # head pairs per core
CHUNK = 512              # q columns per attention pass / gather block
NCH = S // CHUNK         # 4

_CACHE = {}


def _f16(a):
    return np.ascontiguousarray(a, dtype=np.float16)


def _build():
    import concourse.bacc as bacc
    import concourse.mybir as mybir
    import concourse.tile as tile

    f32 = mybir.dt.float32
    f16 = mybir.dt.float16
    Exp = mybir.ActivationFunctionType.Exp

    nc = bacc.Bacc(num_devices=NCORES)
    xT = nc.dram_tensor("xT", [D, S], f16, kind="ExternalInput")
    wqkT = nc.dram_tensor("wqkT", [D, 2 * LCH], f16, kind="ExternalInput")
    wvT = nc.dram_tensor("wvT", [D, LCH], f16, kind="ExternalInput")
    woT = nc.dram_tensor("woT", [D, LCH], f16, kind="ExternalInput")
    mask = nc.dram_tensor("mask", [128, 128], f16, kind="ExternalInput")
    outT = nc.dram_tensor("outT", [LCH, S], f16, kind="ExternalOutput")

    RG = [[0, 1, 2, 3], [4, 5, 6, 7]]

    with tile.TileContext(nc, num_cores=NCORES) as tc:
        with (
            tc.tile_pool(name="const", bufs=1) as const,
            tc.tile_pool(name="qkv", bufs=1) as qkv,
            tc.tile_pool(name="psum", bufs=1, space="PSUM") as psum,
            tc.tile_pool(name="dram", bufs=1, space="DRAM") as dram,
            tc.tile_pool(name="work", bufs=1) as work,
            tc.tile_pool(name="proj", bufs=1) as projp,
            tc.tile_pool(name="agp", bufs=1) as agp,
        ):
            # warmup collective first: absorbs CC-stream ramp + launch skew
            warm_in = dram.tile([128, 8], f16, name="warm_in")
            warm_out = dram.tile([GROUP * 128, 8], f16, name="warm_out")
            nc.sync.dma_start(warm_in[:], mask[:, 0:8])
            nc.gpsimd.collective_compute(
                "AllGather", mybir.AluOpType.bypass, replica_groups=RG,
                ins=[warm_in[:]], outs=[warm_out[:]])

            cc_in = [dram.tile([128, 2 * CHUNK], f16, name=f"ccin{g}")
                     for g in range(NCH)]
            cc_out = [dram.tile([GROUP * 128, 2 * CHUNK], f16, name=f"ccout{g}")
                      for g in range(NCH)]

            mask_sb = const.tile([128, 128], f16)
            ones4 = const.tile([128, LHEADS], f32)
            nc.vector.memset(ones4[:], 1.0)

            qt = qkv.tile([128, PAIRS, S], f16)
            kt = qkv.tile([128, PAIRS, S], f16)
            v = qkv.tile([128, ST, LHEADS, 65], f16)

            # ---------------- input loads ----------------
            wqk = projp.tile([128, KT, 2 * LCH], f16)
            nc.sync.dma_start(wqk[:], wqkT[:].rearrange("(k p) n -> p k n", p=128))
            nc.sync.dma_start(mask_sb[:], mask[:])
            xt = projp.tile([128, KT, S], f16)
            for k in range(KT):
                nc.sync.dma_start(xt[:, k, :], xT[128 * k:128 * k + 128, :])
            wv = projp.tile([128, KT, LCH], f16)
            nc.sync.dma_start(wv[:], wvT[:].rearrange("(k p) n -> p k n", p=128))
            wo = projp.tile([128, KT, LCH], f16)
            nc.sync.dma_start(wo[:], woT[:].rearrange("(k p) n -> p k n", p=128))

            def qk_half(m, half, tag):
                # m: 0,1 = q pair 0/1; 2,3 = k pair 0/1; half = 1024 cols
                dst = qt if m < 2 else kt
                pp = psum.tile([128, 1024], f32, tag=tag, name=f"qk{m}{half}")
                for k in range(KT):
                    for c2 in range(2):
                        o = 1024 * half + 512 * c2
                        nc.tensor.matmul(
                            pp[:, 512 * c2:512 * c2 + 512],
                            wqk[:, k, 128 * m:128 * m + 128],
                            xt[:, k, o:o + 512],
                            start=(k == 0), stop=(k == KT - 1))
                nc.vector.tensor_copy(
                    dst[:, m % 2, 1024 * half:1024 * half + 1024], pp[:])

            def v_proj(j, tag):
                vps = psum.tile([128, LCH], f32, tag=tag, name=f"v{j}")
                for k in range(KT):
                    nc.tensor.matmul(
                        vps[:], xt[:, k, 128 * j:128 * j + 128], wv[:, k, :],
                        start=(k == 0), stop=(k == KT - 1))
                nc.vector.tensor_copy(
                    v[:, j, :, 64:65], ones4[:].rearrange("p (h o) -> p h o", o=1))
                nc.vector.tensor_copy(
                    v[:, j, :, 0:64], vps[:].rearrange("p (h e) -> p h e", h=LHEADS))

            ag = {}
            ccin_last = [None]

            def stage_chunk(p, c, attps):
                """Normalize chunk c's accumulators straight out of PSUM and
                ship to the collective buffer."""
                den = work.tile([65, 2 * CHUNK], f32, tag="den", bufs=2,
                                name=f"den{p}{c}")
                nc.vector.tensor_copy(den[64:65, :], attps[64:65, :])
                rcs = work.tile([64, 16], f32, tag="rcs", bufs=2,
                                name=f"rcs{p}{c}")
                nc.sync.dma_start(rcs[:], den[64:65, :])
                rcr = work.tile([64, 16], f32, tag="rcr", bufs=2,
                                name=f"rcr{p}{c}")
                nc.vector.reciprocal(rcr[:], rcs[:])
                rc0 = work.tile([1, 2 * CHUNK], f32, tag="rc0", bufs=2,
                                name=f"rc0{p}{c}")
                nc.sync.dma_start(rc0[0:1, :], rcr[:])
                for h in range(2):
                    bc = work.tile([64, CHUNK], f32, tag=f"bc{h}", bufs=2,
                                   name=f"bc{p}{c}{h}")
                    nc.gpsimd.partition_broadcast(
                        bc[:], rc0[0:1, CHUNK * h:CHUNK * h + CHUNK])
                    ao = work.tile([64, CHUNK], f16, tag=f"ao{h}", bufs=2,
                                   name=f"ao{p}{c}{h}")
                    nc.vector.tensor_mul(
                        ao[:, :], attps[0:64, CHUNK * h:CHUNK * h + CHUNK],
                        bc[:, :])
                    ccin_last[0] = nc.sync.dma_start(
                        cc_in[c][64 * h:64 * h + 64,
                                 CHUNK * p:CHUNK * p + CHUNK], ao[:, :])

            def gather(g):
                nc.gpsimd.collective_compute(
                    "AllGather", mybir.AluOpType.bypass, replica_groups=RG,
                    ins=[cc_in[g][:]], outs=[cc_out[g][:]])

            def attn_chunk(p, c, gc):
                q0 = CHUNK * c
                nj = 4 * c + 4
                attps = psum.tile([65, 2 * CHUNK], f32,
                                  tag=("at" if gc % 2 == 0 else "pp"),
                                  name=f"att{p}{c}")
                for j in range(nj):
                    qs = max(q0, 128 * j)
                    n = q0 + CHUNK - qs
                    off = qs - q0
                    sc = psum.tile([128, 1024], f32, tag="sc", bufs=2,
                                   name=f"sc{p}{c}{j}")
                    for h in range(2):
                        pb = 64 * h
                        nc.tensor.matmul(
                            sc[:, 512 * h:512 * h + n],
                            kt[pb:pb + 64, p, 128 * j:128 * j + 128],
                            qt[pb:pb + 64, p, qs:qs + n],
                            start=True, stop=True)
                    ex = work.tile([128, 1024], f16, tag="ex", bufs=3,
                                   name=f"ex{p}{c}{j}")
                    if n == CHUNK:
                        nc.scalar.activation(ex[:, :], sc[:, :], Exp)
                    else:
                        nc.scalar.activation(
                            ex[:].rearrange("q (t x) -> q t x", t=2)[:, :, 0:n],
                            sc[:].rearrange("q (t x) -> q t x", t=2)[:, :, 0:n],
                            Exp)
                    if qs == 128 * j:  # diagonal tile: causal mask
                        for h in range(2):
                            nc.vector.tensor_mul(
                                ex[:, 512 * h:512 * h + 128],
                                ex[:, 512 * h:512 * h + 128], mask_sb[:])
                    for h in range(2):
                        nc.tensor.matmul(
                            attps[:, 512 * h + off:512 * h + CHUNK],
                            v[:, j, 2 * p + h, :],
                            ex[:, 512 * h:512 * h + n],
                            start=(j == 0), stop=(j == nj - 1))
                stage_chunk(p, c, attps)

            def prefetch(g):
                # Pin behind the newest staging DMA so the scheduler cannot
                # hoist the AllGather wait ahead of attention staging.
                for r in range(GROUP):
                    t = agp.tile([128, 2 * CHUNK], f16, name=f"ag{g}{r}")
                    dma = nc.sync.dma_start(
                        t[:], cc_out[g][128 * r:128 * r + 128, :])
                    if ccin_last[0] is not None:
                        tile.add_dep_helper(
                            dma.ins, ccin_last[0].ins, sync=True,
                            reason="gather prefetch after staging")
                    ag[(g, r)] = t

            def out_proj(g, tags=("pp", "at")):
                for ct in range(2):
                    pp = psum.tile([128, CHUNK], f32, tag=tags[ct],
                                   name=f"op{g}{ct}")
                    for k in range(KT):
                        nc.tensor.matmul(
                            pp[:], wo[:, k, 128 * ct:128 * ct + 128],
                            ag[(g, k % 4)][:, CHUNK * (k // 4):
                                           CHUNK * (k // 4) + CHUNK],
                            start=(k == 0), stop=(k == KT - 1))
                    ot = agp.tile([128, CHUNK], f16, tag=f"ot{ct}", bufs=2,
                                  name=f"ot{g}{ct}")
                    nc.scalar.copy(ot[:], pp[:])
                    nc.sync.dma_start(
                        outT[128 * ct:128 * ct + 128,
                             CHUNK * g:CHUNK * g + CHUNK], ot[:])

            # ---------------- schedule ----------------
            qk_half(0, 0, "at")
            qk_half(2, 0, "pp")
            v_proj(0, "at")
            v_proj(1, "pp")
            v_proj(2, "at")
            v_proj(3, "pp")
            attn_chunk(0, 0, 0)        # at
            qk_half(1, 0, "pp")
            qk_half(3, 0, "pp")
            attn_chunk(1, 0, 1)        # pp
            gather(0)
            v_proj(4, "at")
            v_proj(5, "at")
            v_proj(6, "at")
            v_proj(7, "at")
            attn_chunk(0, 1, 2)        # at
            attn_chunk(1, 1, 3)        # pp
            gather(1)
            qk_half(0, 1, "pp")
            qk_half(2, 1, "pp")
            v_proj(8, "at")
            v_proj(9, "at")
            v_proj(10, "at")
            v_proj(11, "at")
            attn_chunk(0, 2, 4)        # at
            qk_half(1, 1, "pp")
            qk_half(3, 1, "pp")
            attn_chunk(1, 2, 5)        # pp
            gather(2)
            prefetch(0)
            out_proj(0, ("pp", "at"))
            v_proj(12, "at")
            v_proj(13, "at")
            v_proj(14, "at")
            v_proj(15, "at")
            attn_chunk(0, 3, 6)        # at
            prefetch(1)
            out_proj(1, ("pp", "at"))
            attn_chunk(1, 3, 7)        # pp
            gather(3)
            prefetch(2)
            out_proj(2, ("pp", "at"))
            prefetch(3)
            out_proj(3, ("pp", "at"))

    nc.compile()
    return nc


def _gather_perm():
    """d-channel permutation matching the AllGather layout."""
    perm = []
    for p in range(PAIRS):
        for r in range(GROUP):
            for h in range(2):
                head = 4 * r + 2 * p + h
                perm.extend(range(HD * head, HD * head + HD))
    return np.array(perm)


def _shard_inputs(x, Wq, Wk, Wv, Wo):
    x = np.asarray(x, dtype=np.float32)
    Wq = np.asarray(Wq, dtype=np.float32)
    Wk = np.asarray(Wk, dtype=np.float32)
    Wv = np.asarray(Wv, dtype=np.float32)
    Wo = np.asarray(Wo, dtype=np.float32)
    mask = np.triu(np.ones((128, 128), dtype=np.float16))
    perm = _gather_perm()
    in_maps = []
    for c in range(NCORES):
        b, g = c // GROUP, c % GROUP
        rows = slice(LCH * g, LCH * g + LCH)
        in_maps.append({
            "xT": _f16(x[b].T),
            "wqkT": _f16(np.concatenate([Wq[rows] / 8.0, Wk[rows]], axis=0).T),
            "wvT": _f16(Wv[rows].T),
            "woT": _f16(Wo[rows].T[perm, :]),
            "mask": mask,
        })
    return in_maps


def kernel(x, Wq, Wk, Wv, Wo):
    from concourse.bass_utils import run_bass_kernel_spmd

    if "nc" not in _CACHE:
        _CACHE["nc"] = _build()
    nc = _CACHE["nc"]
    in_maps = _shard_inputs(x, Wq, Wk, Wv, Wo)
    res = run_bass_kernel_spmd(nc, in_maps, core_ids=list(range(NCORES)))
    _CACHE["last_results"] = res
    out = np.empty((B, S, D), dtype=np.float32)
    for c in range(NCORES):
        b, g = c // GROUP, c % GROUP
        out[b][:, LCH * g:LCH * g + LCH] = \
            res.results[c]["outT"].T.astype(np.float32)
    return out


# revision 11
# speedup vs baseline: 1.1754x; 1.0903x over previous
"""Causal self-attention on 8 Trainium2 NeuronCores.

Sharding (data + head parallel): core c handles batch b = c // 4 and the
4 heads [4g, 4g+4) where g = c % 4.  Each core projects q/k/v for its
heads (weights pre-sliced + pre-transposed on host), runs causal
attention, then the 4 cores of each batch AllGather the per-head
attention outputs (hd-major fp16) and each computes a disjoint
256-channel column slice of the output projection.

Schedule notes (v3):
- The two head-pairs are interleaved chunk-by-chunk; one AllGather per
  512-column chunk (covering both pairs) fires as soon as that chunk is
  normalized on both pairs: collectives overlap attention, and only the
  last chunk's (256KB-in) gather is exposed in the tail.
- Output projection runs per 512-column group as soon as its gather has
  landed, filling tensor-engine slack in the ACT(exp)-paced attention
  loop; only the last group is in the tail.
- q/k projections are emitted in 1024-column halves interleaved with the
  first chunks so the exp stream starts ~20us into the kernel.
- Softmax normalization: the 1024 denominators of a chunk are copied out
  of PSUM as one row, reshaped across 64 partitions by a small DMA,
  inverted with DVE reciprocal (serial-per-partition: ~16 elements each
  instead of 1024), reshaped back, broadcast, and multiplied into the
  PSUM accumulators directly.
- A tiny warmup AllGather at kernel start absorbs the CC-stream
  first-collective ramp and cross-core launch skew.
- fp16 data path, fp32 PSUM accumulation, fp16 output (absmax ~4).
- PSUM budget (8 banks): score tile 2 banks x 2 bufs; "at"/"pp" spaces
  2 banks each, shared by attention accumulators (alternating),
  projection tiles, and output-projection tiles in emission order.

Layouts per core:
  xT    (1024, 2048)  x[b].T                       (d on partitions)
  wqkT  (1024, 512)   [ (Wq[rows]/8).T | Wk[rows].T ]
  wvT   (1024, 256)   Wv[rows].T
  woT   (1024, 256)   Wo[rows].T with rows permuted to the AllGather
                      order: [pair p=0: rank r: heads 4r,4r+1] then
                      [pair p=1: rank r: heads 4r+2,4r+3]
  mask  (128, 128)    upper-triangular ones (k <= q)
  outT  (256, 2048)   out[b][:, cols].T  (fp16)
"""

import numpy as np

B, S, D, H = 2, 2048, 1024, 16
HD = D // H              # 64
NCORES = 8
GROUP = 4                # cores per batch
LHEADS = 4               # heads per core
LCH = LHEADS * HD        # 256 local channels
KT = D // 128            # 8 contraction tiles
ST = S // 128            # 16 sequence tiles
PAIRS = 2                # head pairs per core
CHUNK = 512              # q columns per attention pass / gather block
NCH = S // CHUNK         # 4

_CACHE = {}


def _f16(a):
    return np.ascontiguousarray(a, dtype=np.float16)


def _build():
    import concourse.bacc as bacc
    import concourse.mybir as mybir
    import concourse.tile as tile

    f32 = mybir.dt.float32
    f16 = mybir.dt.float16
    Exp = mybir.ActivationFunctionType.Exp

    nc = bacc.Bacc(num_devices=NCORES)
    xT = nc.dram_tensor("xT", [D, S], f16, kind="ExternalInput")
    wqkT = nc.dram_tensor("wqkT", [D, 2 * LCH], f16, kind="ExternalInput")
    wvT = nc.dram_tensor("wvT", [D, LCH], f16, kind="ExternalInput")
    woT = nc.dram_tensor("woT", [D, LCH], f16, kind="ExternalInput")
    mask = nc.dram_tensor("mask", [128, 128], f16, kind="ExternalInput")
    outT = nc.dram_tensor("outT", [LCH, S], f16, kind="ExternalOutput")

    RG = [[0, 1, 2, 3], [4, 5, 6, 7]]

    with tile.TileContext(nc, num_cores=NCORES) as tc:
        with (
            tc.tile_pool(name="const", bufs=1) as const,
            tc.tile_pool(name="qkv", bufs=1) as qkv,
            tc.tile_pool(name="psum", bufs=1, space="PSUM") as psum,
            tc.tile_pool(name="dram", bufs=1, space="DRAM") as dram,
            tc.tile_pool(name="work", bufs=1) as work,
            tc.tile_pool(name="proj", bufs=1) as projp,
            tc.tile_pool(name="agp", bufs=1) as agp,
        ):
            # warmup collective first: absorbs CC-stream ramp + launch skew
            warm_in = dram.tile([128, 8], f16, name="warm_in")
            warm_out = dram.tile([GROUP * 128, 8], f16, name="warm_out")
            nc.sync.dma_start(warm_in[:], mask[:, 0:8])
            nc.gpsimd.collective_compute(
                "AllGather", mybir.AluOpType.bypass, replica_groups=RG,
                ins=[warm_in[:]], outs=[warm_out[:]])

            cc_in = [dram.tile([128, 2 * CHUNK], f16, name=f"ccin{g}")
                     for g in range(NCH)]
            cc_out = [dram.tile([GROUP * 128, 2 * CHUNK], f16, name=f"ccout{g}")
                      for g in range(NCH)]

            mask_sb = const.tile([128, 128], f16)
            ones4 = const.tile([128, LHEADS], f32)
            nc.vector.memset(ones4[:], 1.0)

            qt = qkv.tile([128, PAIRS, S], f16)
            kt = qkv.tile([128, PAIRS, S], f16)
            v = qkv.tile([128, ST, LHEADS, 65], f16)

            # ---------------- input loads ----------------
            wqk = projp.tile([128, KT, 2 * LCH], f16)
            nc.sync.dma_start(wqk[:], wqkT[:].rearrange("(k p) n -> p k n", p=128))
            nc.sync.dma_start(mask_sb[:], mask[:])
            # xt as KT separate tiles: per-k DMA completion unblocks that
            # k-tile's matmuls (a single merged tile would make the first
            # matmul wait on the whole 4MB load)
            xt = []
            for k in range(KT):
                tx = projp.tile([128, S], f16, name=f"xt{k}")
                nc.sync.dma_start(tx[:], xT[128 * k:128 * k + 128, :])
                xt.append(tx)
            wv = projp.tile([128, KT, LCH], f16)
            nc.sync.dma_start(wv[:], wvT[:].rearrange("(k p) n -> p k n", p=128))
            wo = projp.tile([128, KT, LCH], f16)
            nc.sync.dma_start(wo[:], woT[:].rearrange("(k p) n -> p k n", p=128))

            def qk_half(m, half, tag):
                # m: 0,1 = q pair 0/1; 2,3 = k pair 0/1; half = 1024 cols
                dst = qt if m < 2 else kt
                pp = psum.tile([128, 1024], f32, tag=tag, name=f"qk{m}{half}")
                for k in range(KT):
                    for c2 in range(2):
                        o = 1024 * half + 512 * c2
                        nc.tensor.matmul(
                            pp[:, 512 * c2:512 * c2 + 512],
                            wqk[:, k, 128 * m:128 * m + 128],
                            xt[k][:, o:o + 512],
                            start=(k == 0), stop=(k == KT - 1))
                nc.vector.tensor_copy(
                    dst[:, m % 2, 1024 * half:1024 * half + 1024], pp[:])

            def v_proj(j, tag):
                vps = psum.tile([128, LCH], f32, tag=tag, name=f"v{j}")
                for k in range(KT):
                    nc.tensor.matmul(
                        vps[:], xt[k][:, 128 * j:128 * j + 128], wv[:, k, :],
                        start=(k == 0), stop=(k == KT - 1))
                nc.vector.tensor_copy(
                    v[:, j, :, 64:65], ones4[:].rearrange("p (h o) -> p h o", o=1))
                nc.vector.tensor_copy(
                    v[:, j, :, 0:64], vps[:].rearrange("p (h e) -> p h e", h=LHEADS))

            ag = {}
            ccin_last = [None]

            def stage_chunk(p, c, attps):
                """Copy the accumulators out of PSUM in one shot (frees the
                at/pp space for the next same-tag user immediately), then
                normalize from SBUF and ship to the collective buffer."""
                asb = work.tile([65, 2 * CHUNK], f32, tag="asb", bufs=2,
                                name=f"asb{p}{c}")
                nc.vector.tensor_copy(asb[:], attps[:])
                rcs = work.tile([64, 16], f32, tag="rcs", bufs=2,
                                name=f"rcs{p}{c}")
                nc.sync.dma_start(rcs[:], asb[64:65, :])
                rcr = work.tile([64, 16], f32, tag="rcr", bufs=2,
                                name=f"rcr{p}{c}")
                nc.vector.reciprocal(rcr[:], rcs[:])
                rc0 = work.tile([1, 2 * CHUNK], f32, tag="rc0", bufs=2,
                                name=f"rc0{p}{c}")
                nc.sync.dma_start(rc0[0:1, :], rcr[:])
                for h in range(2):
                    bc = work.tile([64, CHUNK], f32, tag=f"bc{h}", bufs=2,
                                   name=f"bc{p}{c}{h}")
                    nc.gpsimd.partition_broadcast(
                        bc[:], rc0[0:1, CHUNK * h:CHUNK * h + CHUNK])
                    ao = work.tile([64, CHUNK], f16, tag=f"ao{h}", bufs=2,
                                   name=f"ao{p}{c}{h}")
                    nc.vector.tensor_mul(
                        ao[:, :], asb[0:64, CHUNK * h:CHUNK * h + CHUNK],
                        bc[:, :])
                    ccin_last[0] = nc.sync.dma_start(
                        cc_in[c][64 * h:64 * h + 64,
                                 CHUNK * p:CHUNK * p + CHUNK], ao[:, :])

            def gather(g):
                nc.gpsimd.collective_compute(
                    "AllGather", mybir.AluOpType.bypass, replica_groups=RG,
                    ins=[cc_in[g][:]], outs=[cc_out[g][:]])

            def attn_chunk(p, c, gc):
                q0 = CHUNK * c
                nj = 4 * c + 4
                attps = psum.tile([65, 2 * CHUNK], f32,
                                  tag=("at" if gc % 2 == 0 else "pp"),
                                  name=f"att{p}{c}")
                for j in range(nj):
                    qs = max(q0, 128 * j)
                    n = q0 + CHUNK - qs
                    off = qs - q0
                    sc = psum.tile([128, 1024], f32, tag="sc", bufs=2,
                                   name=f"sc{p}{c}{j}")
                    for h in range(2):
                        pb = 64 * h
                        nc.tensor.matmul(
                            sc[:, 512 * h:512 * h + n],
                            kt[pb:pb + 64, p, 128 * j:128 * j + 128],
                            qt[pb:pb + 64, p, qs:qs + n],
                            start=True, stop=True)
                    ex = work.tile([128, 1024], f16, tag="ex", bufs=3,
                                   name=f"ex{p}{c}{j}")
                    if n == CHUNK:
                        nc.scalar.activation(ex[:, :], sc[:, :], Exp)
                    else:
                        nc.scalar.activation(
                            ex[:].rearrange("q (t x) -> q t x", t=2)[:, :, 0:n],
                            sc[:].rearrange("q (t x) -> q t x", t=2)[:, :, 0:n],
                            Exp)
                    if qs == 128 * j:  # diagonal tile: causal mask
                        for h in range(2):
                            nc.vector.tensor_mul(
                                ex[:, 512 * h:512 * h + 128],
                                ex[:, 512 * h:512 * h + 128], mask_sb[:])
                    for h in range(2):
                        nc.tensor.matmul(
                            attps[:, 512 * h + off:512 * h + CHUNK],
                            v[:, j, 2 * p + h, :],
                            ex[:, 512 * h:512 * h + n],
                            start=(j == 0), stop=(j == nj - 1))
                stage_chunk(p, c, attps)

            def prefetch(g):
                # Pin behind the newest staging DMA so the scheduler cannot
                # hoist the AllGather wait ahead of attention staging.
                for r in range(GROUP):
                    t = agp.tile([128, 2 * CHUNK], f16, name=f"ag{g}{r}")
                    dma = nc.sync.dma_start(
                        t[:], cc_out[g][128 * r:128 * r + 128, :])
                    if ccin_last[0] is not None:
                        tile.add_dep_helper(
                            dma.ins, ccin_last[0].ins, sync=True,
                            reason="gather prefetch after staging")
                    ag[(g, r)] = t

            def out_proj(g, tags=("pp", "at")):
                for ct in range(2):
                    pp = psum.tile([128, CHUNK], f32, tag=tags[ct],
                                   name=f"op{g}{ct}")
                    for k in range(KT):
                        nc.tensor.matmul(
                            pp[:], wo[:, k, 128 * ct:128 * ct + 128],
                            ag[(g, k % 4)][:, CHUNK * (k // 4):
                                           CHUNK * (k // 4) + CHUNK],
                            start=(k == 0), stop=(k == KT - 1))
                    ot = agp.tile([128, CHUNK], f16, tag=f"ot{ct}", bufs=2,
                                  name=f"ot{g}{ct}")
                    nc.scalar.copy(ot[:], pp[:])
                    nc.sync.dma_start(
                        outT[128 * ct:128 * ct + 128,
                             CHUNK * g:CHUNK * g + CHUNK], ot[:])

            # ---------------- schedule ----------------
            qk_half(0, 0, "at")
            qk_half(2, 0, "pp")
            v_proj(0, "at")
            v_proj(1, "pp")
            v_proj(2, "at")
            v_proj(3, "pp")
            attn_chunk(0, 0, 0)        # at
            qk_half(1, 0, "pp")
            qk_half(3, 0, "pp")
            attn_chunk(1, 0, 1)        # pp
            gather(0)
            v_proj(4, "at")
            v_proj(5, "pp")
            v_proj(6, "at")
            v_proj(7, "pp")
            attn_chunk(0, 1, 2)        # at
            attn_chunk(1, 1, 3)        # pp
            gather(1)
            qk_half(0, 1, "pp")
            qk_half(2, 1, "at")
            v_proj(8, "pp")
            v_proj(9, "at")
            v_proj(10, "pp")
            v_proj(11, "at")
            attn_chunk(0, 2, 4)        # at
            qk_half(1, 1, "pp")
            qk_half(3, 1, "pp")
            attn_chunk(1, 2, 5)        # pp
            gather(2)
            prefetch(0)
            out_proj(0, ("pp", "at"))
            v_proj(12, "pp")
            v_proj(13, "at")
            v_proj(14, "pp")
            v_proj(15, "at")
            attn_chunk(0, 3, 6)        # at
            prefetch(1)
            out_proj(1, ("pp", "at"))
            attn_chunk(1, 3, 7)        # pp
            gather(3)
            prefetch(2)
            out_proj(2, ("pp", "at"))
            prefetch(3)
            out_proj(3, ("pp", "at"))

    nc.compile()
    return nc


def _gather_perm():
    """d-channel permutation matching the AllGather layout."""
    perm = []
    for p in range(PAIRS):
        for r in range(GROUP):
            for h in range(2):
                head = 4 * r + 2 * p + h
                perm.extend(range(HD * head, HD * head + HD))
    return np.array(perm)


def _shard_inputs(x, Wq, Wk, Wv, Wo):
    x = np.asarray(x, dtype=np.float32)
    Wq = np.asarray(Wq, dtype=np.float32)
    Wk = np.asarray(Wk, dtype=np.float32)
    Wv = np.asarray(Wv, dtype=np.float32)
    Wo = np.asarray(Wo, dtype=np.float32)
    mask = np.triu(np.ones((128, 128), dtype=np.float16))
    perm = _gather_perm()
    in_maps = []
    for c in range(NCORES):
        b, g = c // GROUP, c % GROUP
        rows = slice(LCH * g, LCH * g + LCH)
        in_maps.append({
            "xT": _f16(x[b].T),
            "wqkT": _f16(np.concatenate([Wq[rows] / 8.0, Wk[rows]], axis=0).T),
            "wvT": _f16(Wv[rows].T),
            "woT": _f16(Wo[rows].T[perm, :]),
            "mask": mask,
        })
    return in_maps


def kernel(x, Wq, Wk, Wv, Wo):
    from concourse.bass_utils import run_bass_kernel_spmd

    if "nc" not in _CACHE:
        _CACHE["nc"] = _build()
    nc = _CACHE["nc"]
    in_maps = _shard_inputs(x, Wq, Wk, Wv, Wo)
    res = run_bass_kernel_spmd(nc, in_maps, core_ids=list(range(NCORES)))
    _CACHE["last_results"] = res
    out = np.empty((B, S, D), dtype=np.float32)
    for c in range(NCORES):
        b, g = c // GROUP, c % GROUP
        out[b][:, LCH * g:LCH * g + LCH] = \
            res.results[c]["outT"].T.astype(np.float32)
    return out


# revision 15
# speedup vs baseline: 1.3033x; 1.1089x over previous
"""Causal self-attention on 8 Trainium2 NeuronCores.

Sharding (data + head parallel): core c handles batch b = c // 4 and the
4 heads [4g, 4g+4) where g = c % 4.  Each core projects q/k/v for its
heads (weights pre-sliced + pre-transposed on host), runs causal
attention, then the 4 cores of each batch AllGather the per-head
attention outputs (hd-major fp16) and each computes a disjoint
256-channel column slice of the output projection.

Schedule notes (v4):
- The attention inner loop is exp(ACT)-paced (~1us per 128-key step);
  projection work (q/k second halves, v tiles, output projection) is
  emitted as small self-contained "filler" closures drained one per
  j-step inside the attention loops, so the in-order PE queue never
  parks a multi-us block in front of the next score matmul.
- One AllGather per 512-column chunk (both head pairs) fires as soon as
  the chunk is staged on both pairs; gather g's SBUF prefetch is issued
  about one chunk later, and its out_proj slice runs as filler inside a
  later chunk.  Only the final chunk's gather + out_proj sit in the
  tail.
- Dummy matmuls on the first-loaded weight tile warm the PE clock (HAM
  K=8/8) while x streams in, and again while the last gather flies so
  the tail out_proj runs at 2.4GHz.
- Softmax normalization: accumulators leave PSUM in one copy (frees the
  at/pp space immediately); the 1024 denominators are reshaped across
  64 partitions by DMA, inverted (DVE reciprocal is serial per
  partition), reshaped back, broadcast once to [64,1024], and applied
  with a single multiply; one DMA ships both heads to the collective
  buffer.
- A tiny warmup AllGather at kernel start absorbs CC-stream ramp and
  cross-core launch skew.
- fp16 data path, fp32 PSUM accumulation, fp16 output (absmax ~4).
- PSUM budget (8 banks): score tile 2 banks x 2 bufs; "at"/"pp" spaces
  2 banks each, alternating between attention accumulators and
  filler projection tiles (fillers always use the opposite space of
  the chunk they are drained into).

Layouts per core:
  xT    (1024, 2048)  x[b].T                       (d on partitions)
  wqkT  (1024, 512)   [ (Wq[rows]/8).T | Wk[rows].T ]
  wvT   (1024, 256)   Wv[rows].T
  woT   (1024, 256)   Wo[rows].T with rows permuted to the AllGather
                      order: [pair p=0: rank r: heads 4r,4r+1] then
                      [pair p=1: rank r: heads 4r+2,4r+3]
  mask  (128, 128)    upper-triangular ones (k <= q)
  outT  (256, 2048)   out[b][:, cols].T  (fp16)
"""

from collections import deque

import numpy as np

B, S, D, H = 2, 2048, 1024, 16
HD = D // H              # 64
NCORES = 8
GROUP = 4                # cores per batch
LHEADS = 4               # heads per core
LCH = LHEADS * HD        # 256 local channels
KT = D // 128            # 8 contraction tiles
ST = S // 128            # 16 sequence tiles
PAIRS = 2                # head pairs per core
CHUNK = 512              # q columns per attention pass / gather block
NCH = S // CHUNK         # 4

_CACHE = {}


def _f16(a):
    return np.ascontiguousarray(a, dtype=np.float16)


def _build():
    import concourse.bacc as bacc
    import concourse.mybir as mybir
    import concourse.tile as tile

    f32 = mybir.dt.float32
    f16 = mybir.dt.float16
    Exp = mybir.ActivationFunctionType.Exp

    nc = bacc.Bacc(num_devices=NCORES)
    xT = nc.dram_tensor("xT", [D, S], f16, kind="ExternalInput")
    wqkT = nc.dram_tensor("wqkT", [D, 2 * LCH], f16, kind="ExternalInput")
    wvT = nc.dram_tensor("wvT", [D, LCH], f16, kind="ExternalInput")
    woT = nc.dram_tensor("woT", [D, LCH], f16, kind="ExternalInput")
    mask = nc.dram_tensor("mask", [128, 128], f16, kind="ExternalInput")
    outT = nc.dram_tensor("outT", [LCH, S], f16, kind="ExternalOutput")

    RG = [[0, 1, 2, 3], [4, 5, 6, 7]]

    with tile.TileContext(nc, num_cores=NCORES) as tc:
        with (
            tc.tile_pool(name="const", bufs=1) as const,
            tc.tile_pool(name="qkv", bufs=1) as qkv,
            tc.tile_pool(name="psum", bufs=1, space="PSUM") as psum,
            tc.tile_pool(name="dram", bufs=1, space="DRAM") as dram,
            tc.tile_pool(name="work", bufs=1) as work,
            tc.tile_pool(name="proj", bufs=1) as projp,
            tc.tile_pool(name="agp", bufs=1) as agp,
        ):
            # warmup collective first: absorbs CC-stream ramp + launch skew
            warm_in = dram.tile([128, 8], f16, name="warm_in")
            warm_out = dram.tile([GROUP * 128, 8], f16, name="warm_out")
            nc.sync.dma_start(warm_in[:], mask[:, 0:8])
            nc.gpsimd.collective_compute(
                "AllGather", mybir.AluOpType.bypass, replica_groups=RG,
                ins=[warm_in[:]], outs=[warm_out[:]])

            cc_in = [dram.tile([128, 2 * CHUNK], f16, name=f"ccin{g}")
                     for g in range(NCH)]
            cc_out = [dram.tile([GROUP * 128, 2 * CHUNK], f16, name=f"ccout{g}")
                      for g in range(NCH)]

            mask_sb = const.tile([128, 128], f16)
            ones4 = const.tile([128, LHEADS], f32)
            nc.vector.memset(ones4[:], 1.0)

            qt = qkv.tile([128, PAIRS, S], f16)
            kt = qkv.tile([128, PAIRS, S], f16)
            v = qkv.tile([128, ST, LHEADS, 65], f16)

            # ---------------- input loads ----------------
            wqk = projp.tile([128, KT, 2 * LCH], f16)
            nc.sync.dma_start(wqk[:], wqkT[:].rearrange("(k p) n -> p k n", p=128))
            nc.sync.dma_start(mask_sb[:], mask[:])
            xt = []
            for k in range(KT):
                tx = projp.tile([128, S], f16, name=f"xt{k}")
                nc.sync.dma_start(tx[:], xT[128 * k:128 * k + 128, :])
                xt.append(tx)
            wv = projp.tile([128, KT, LCH], f16)
            nc.sync.dma_start(wv[:], wvT[:].rearrange("(k p) n -> p k n", p=128))
            wo = projp.tile([128, KT, LCH], f16)
            nc.sync.dma_start(wo[:], woT[:].rearrange("(k p) n -> p k n", p=128))

            def warm_pe(n, name, tag):
                """Dummy matmuls on the wqk tile: keep the PE HAM warm while
                it would otherwise idle (startup x-load, tail gather wait)."""
                junk = psum.tile([128, 512], f32, tag=tag, name=name)
                for i in range(n):
                    nc.tensor.matmul(
                        junk[:], wqk[:, 0, 0:128], wqk[:, i % KT, 0:512],
                        start=True, stop=True)

            warm_pe(20, "warmup", "at")

            def qk_half(m, half, tag):
                # m: 0,1 = q pair 0/1; 2,3 = k pair 0/1; half = 1024 cols
                dst = qt if m < 2 else kt
                pp = psum.tile([128, 1024], f32, tag=tag, name=f"qk{m}{half}")
                for k in range(KT):
                    for c2 in range(2):
                        o = 1024 * half + 512 * c2
                        nc.tensor.matmul(
                            pp[:, 512 * c2:512 * c2 + 512],
                            wqk[:, k, 128 * m:128 * m + 128],
                            xt[k][:, o:o + 512],
                            start=(k == 0), stop=(k == KT - 1))
                nc.vector.tensor_copy(
                    dst[:, m % 2, 1024 * half:1024 * half + 1024], pp[:])

            def v_proj(j, tag):
                vps = psum.tile([128, LCH], f32, tag=tag, name=f"v{j}")
                for k in range(KT):
                    nc.tensor.matmul(
                        vps[:], xt[k][:, 128 * j:128 * j + 128], wv[:, k, :],
                        start=(k == 0), stop=(k == KT - 1))
                nc.vector.tensor_copy(
                    v[:, j, :, 64:65], ones4[:].rearrange("p (h o) -> p h o", o=1))
                nc.vector.tensor_copy(
                    v[:, j, :, 0:64], vps[:].rearrange("p (h e) -> p h e", h=LHEADS))

            ag = {}
            ccin_last = [None]

            def stage_chunk(p, c, attps):
                """Copy the accumulators out of PSUM in one shot (frees the
                at/pp space immediately), then normalize from SBUF and ship
                both heads to the collective buffer in one DMA."""
                asb = work.tile([65, 2 * CHUNK], f32, tag="asb", bufs=2,
                                name=f"asb{p}{c}")
                nc.vector.tensor_copy(asb[:], attps[:])
                rcs = work.tile([64, 16], f32, tag="rcs", bufs=2,
                                name=f"rcs{p}{c}")
                nc.sync.dma_start(rcs[:], asb[64:65, :])
                rcr = work.tile([64, 16], f32, tag="rcr", bufs=2,
                                name=f"rcr{p}{c}")
                nc.vector.reciprocal(rcr[:], rcs[:])
                rc0 = work.tile([1, 2 * CHUNK], f32, tag="rc0", bufs=2,
                                name=f"rc0{p}{c}")
                nc.sync.dma_start(rc0[0:1, :], rcr[:])
                bc = work.tile([64, 2 * CHUNK], f32, tag="bc", bufs=2,
                               name=f"bc{p}{c}")
                nc.gpsimd.partition_broadcast(bc[:], rc0[0:1, :])
                ao = work.tile([64, 2 * CHUNK], f16, tag="ao", bufs=2,
                               name=f"ao{p}{c}")
                nc.vector.tensor_mul(ao[:, :], asb[0:64, :], bc[:, :])
                # both heads in one DMA: src col 512h+i -> dst row 64h+q
                dst = cc_in[c][:].rearrange("(h q) n -> q h n", h=2)
                ccin_last[0] = nc.sync.dma_start(
                    dst[:, :, CHUNK * p:CHUNK * p + CHUNK], ao[:, :])

            def gather(g):
                nc.gpsimd.collective_compute(
                    "AllGather", mybir.AluOpType.bypass, replica_groups=RG,
                    ins=[cc_in[g][:]], outs=[cc_out[g][:]])

            def prefetch(g):
                # Pin behind the newest staging DMA so the scheduler cannot
                # hoist the AllGather wait ahead of attention staging.
                for r in range(GROUP):
                    t = agp.tile([128, 2 * CHUNK], f16, name=f"ag{g}{r}")
                    dma = nc.sync.dma_start(
                        t[:], cc_out[g][128 * r:128 * r + 128, :])
                    if ccin_last[0] is not None:
                        tile.add_dep_helper(
                            dma.ins, ccin_last[0].ins, sync=True,
                            reason="gather prefetch after staging")
                    ag[(g, r)] = t

            def out_ct(g, ct, tag):
                pp = psum.tile([128, CHUNK], f32, tag=tag, name=f"op{g}{ct}")
                for k in range(KT):
                    nc.tensor.matmul(
                        pp[:], wo[:, k, 128 * ct:128 * ct + 128],
                        ag[(g, k % 4)][:, CHUNK * (k // 4):
                                       CHUNK * (k // 4) + CHUNK],
                        start=(k == 0), stop=(k == KT - 1))
                ot = agp.tile([128, CHUNK], f16, tag=f"ot{ct}", bufs=2,
                              name=f"ot{g}{ct}")
                nc.scalar.copy(ot[:], pp[:])
                nc.sync.dma_start(
                    outT[128 * ct:128 * ct + 128,
                         CHUNK * g:CHUNK * g + CHUNK], ot[:])

            # Filler queue: self-contained closures (own PSUM tile, tag
            # passed at drain time = opposite of the running chunk's attps).
            FILL = deque()

            def drain_all():
                while FILL:
                    FILL.popleft()("pp")   # between chunks: either tag works

            def attn_chunk(p, c, gc):
                opp = "pp" if gc % 2 == 0 else "at"
                q0 = CHUNK * c
                nj = 4 * c + 4
                attps = psum.tile([65, 2 * CHUNK], f32,
                                  tag=("at" if gc % 2 == 0 else "pp"),
                                  name=f"att{p}{c}")
                for j in range(nj):
                    qs = max(q0, 128 * j)
                    n = q0 + CHUNK - qs
                    off = qs - q0
                    sc = psum.tile([128, 1024], f32, tag="sc", bufs=2,
                                   name=f"sc{p}{c}{j}")
                    for h in range(2):
                        pb = 64 * h
                        nc.tensor.matmul(
                            sc[:, 512 * h:512 * h + n],
                            kt[pb:pb + 64, p, 128 * j:128 * j + 128],
                            qt[pb:pb + 64, p, qs:qs + n],
                            start=True, stop=True)
                    ex = work.tile([128, 1024], f16, tag="ex", bufs=3,
                                   name=f"ex{p}{c}{j}")
                    if n == CHUNK:
                        nc.scalar.activation(ex[:, :], sc[:, :], Exp)
                    else:
                        nc.scalar.activation(
                            ex[:].rearrange("q (t x) -> q t x", t=2)[:, :, 0:n],
                            sc[:].rearrange("q (t x) -> q t x", t=2)[:, :, 0:n],
                            Exp)
                    if qs == 128 * j:  # diagonal tile: causal mask
                        for h in range(2):
                            nc.vector.tensor_mul(
                                ex[:, 512 * h:512 * h + 128],
                                ex[:, 512 * h:512 * h + 128], mask_sb[:])
                    for h in range(2):
                        nc.tensor.matmul(
                            attps[:, 512 * h + off:512 * h + CHUNK],
                            v[:, j, 2 * p + h, :],
                            ex[:, 512 * h:512 * h + n],
                            start=(j == 0), stop=(j == nj - 1))
                    if (FILL and j < nj - 1 and j % 2 == 1
                            and (j >= 3 or nj <= 6)):
                        FILL.popleft()(opp)
                stage_chunk(p, c, attps)

            # ---------------- schedule ----------------
            qk_half(0, 0, "at")
            qk_half(2, 0, "pp")
            v_proj(0, "at")
            v_proj(1, "pp")
            v_proj(2, "at")
            v_proj(3, "pp")
            for j in range(4, 8):
                FILL.append(lambda t, j=j: v_proj(j, t))
            attn_chunk(0, 0, 0)        # attps at
            qk_half(1, 0, "pp")
            qk_half(3, 0, "pp")
            attn_chunk(1, 0, 1)        # attps pp
            gather(0)
            drain_all()                # v4-7 must exist before chunk 1
            for m in (0, 2):
                FILL.append(lambda t, m=m: qk_half(m, 1, t))
            for j in range(8, 12):
                FILL.append(lambda t, j=j: v_proj(j, t))
            attn_chunk(0, 1, 2)        # at
            attn_chunk(1, 1, 3)        # pp
            gather(1)
            prefetch(0)
            drain_all()                # q/k half1 (pairs 0) + v8-11 ready
            for m in (1, 3):
                FILL.append(lambda t, m=m: qk_half(m, 1, t))
            attn_chunk(0, 2, 4)        # at
            drain_all()                # q/k half1 (pair 1) before chunk(1,2)
            for j in range(12, 16):
                FILL.append(lambda t, j=j: v_proj(j, t))
            FILL.append(lambda t: out_ct(0, 0, t))
            FILL.append(lambda t: out_ct(0, 1, t))
            attn_chunk(1, 2, 5)        # pp
            gather(2)
            prefetch(1)
            drain_all()                # v12-15 before chunk 3
            FILL.append(lambda t: out_ct(1, 0, t))
            FILL.append(lambda t: out_ct(1, 1, t))
            attn_chunk(0, 3, 6)        # at
            prefetch(2)
            FILL.append(lambda t: out_ct(2, 0, t))
            FILL.append(lambda t: out_ct(2, 1, t))
            attn_chunk(1, 3, 7)        # pp
            gather(3)
            drain_all()
            warm_pe(24, "tailwarm", "pp")  # keep PE warm while gather 3 flies
            prefetch(3)
            out_ct(3, 0, "pp")
            out_ct(3, 1, "at")

    nc.compile()
    return nc


def _gather_perm():
    """d-channel permutation matching the AllGather layout."""
    perm = []
    for p in range(PAIRS):
        for r in range(GROUP):
            for h in range(2):
                head = 4 * r + 2 * p + h
                perm.extend(range(HD * head, HD * head + HD))
    return np.array(perm)


def _shard_inputs(x, Wq, Wk, Wv, Wo):
    x = np.asarray(x, dtype=np.float32)
    Wq = np.asarray(Wq, dtype=np.float32)
    Wk = np.asarray(Wk, dtype=np.float32)
    Wv = np.asarray(Wv, dtype=np.float32)
    Wo = np.asarray(Wo, dtype=np.float32)
    mask = np.triu(np.ones((128, 128), dtype=np.float16))
    perm = _gather_perm()
    in_maps = []
    for c in range(NCORES):
        b, g = c // GROUP, c % GROUP
        rows = slice(LCH * g, LCH * g + LCH)
        in_maps.append({
            "xT": _f16(x[b].T),
            "wqkT": _f16(np.concatenate([Wq[rows] / 8.0, Wk[rows]], axis=0).T),
            "wvT": _f16(Wv[rows].T),
            "woT": _f16(Wo[rows].T[perm, :]),
            "mask": mask,
        })
    return in_maps


def kernel(x, Wq, Wk, Wv, Wo):
    from concourse.bass_utils import run_bass_kernel_spmd

    if "nc" not in _CACHE:
        _CACHE["nc"] = _build()
    nc = _CACHE["nc"]
    in_maps = _shard_inputs(x, Wq, Wk, Wv, Wo)
    res = run_bass_kernel_spmd(nc, in_maps, core_ids=list(range(NCORES)))
    _CACHE["last_results"] = res
    out = np.empty((B, S, D), dtype=np.float32)
    for c in range(NCORES):
        b, g = c // GROUP, c % GROUP
        out[b][:, LCH * g:LCH * g + LCH] = \
            res.results[c]["outT"].T.astype(np.float32)
    return out
